# revision 28
# baseline (speedup 1.0000x reference)
"""Dilated-attention (segmented FlashMHA) for Trainium2, 8-core data parallel.

Problem (hardcoded): x [2, 8192, 1024], SEGMENT=2048, DILATION=2, 16 heads.
Each (batch, segment) pair is an independent attention problem over the
L = 1024 dilated tokens; there are exactly B * n_seg = 2 * 4 = 8 of them,
one per NeuronCore.  Weights are replicated.

v7: the attention pipeline is rebuilt around three
HW-measured facts that the cost model misses:
  1. PSUM reads are the scarce resource: ~550-614 ns per [128,512] f32
     bank from ACT or DVE, and reads from *different* engines serialize
     on a shared path (splitting evictions across engines gains nothing).
     So each slot's score pair lands in ONE 2-bank [128,1024] PSUM tile
     and leaves through ONE wide ACT exp (~1.0 us) - ACT is the only
     PSUM reader in the attention steady state.
  2. A blocked semaphore wait costs ~300 ns to resume, so the busiest
     engine must never block.  The PE (~1.5 us/slot incl. fillers) runs
     against a 2-slot sps window (ps2 bufs=2), 5-slot ctx lag, 10-deep
     et2 pool; ACT and DVE absorb the resume penalties instead.
  3. gpsimd (Pool) ops cost ~6 us each and a [1,512] single-partition
     reciprocal ~3 us - both are banished from the softmax normalize.
     vaug columns 64:128 are all-ones, so the ctx matmul replicates the
     softmax denominator across cps partitions 64:128 for free (a PE
     partition-broadcast); normalize is then: one bf16 ACT eviction of
     the values + one f32 ACT eviction of the denominators + DVE
     reciprocal_approx_fast + one DVE mult, pumped >=2 slots apart so
     nothing ever waits.
  k-tiles skip their bias exactly (softmax is shift-invariant along the
  query axis: only (q+bq)-k survives; k-bk and bq-bk cancel), which
  turns their eviction into a biasless ACT copy.

Per-core kernel phases:
  chase   pair-0 q/k tiles accumulate as xsT DMA chunks land
  attn    128 half-slots (pair, half, c): score pair (row-group paired
          K=64 matmuls, ~161 ns) -> wide exp -> 2 ctx matmuls, with the
          qk/v projection units interleaved as PE fillers per `plan`
  out     ctxT.T-contract @ Wout + bout, lead-in interleaved with the
          drain so the PE never idles on the last normalize
"""

from contextlib import ExitStack

import numpy as np
import ml_dtypes

from concourse import bacc, bass_utils, mybir, tile
from concourse._compat import with_exitstack

F32 = mybir.dt.float32
BF16 = mybir.dt.bfloat16
AF = mybir.ActivationFunctionType

B = 2
S = 8192
D = 1024
SEGMENT = 2048
DILATION = 2
N_SEG = S // SEGMENT          # 4
L = SEGMENT // DILATION       # 1024 tokens per (b, seg)
H = 16
HD = 64
NQK = 2048
SCALE = 0.125                 # 1 / sqrt(HD)
N_CORES = 8
LAG = 5                       # ctx trails scores by LAG half-slots
L1 = 2                        # head-B exp (from SBUF) trails scores by L1

_CACHE = {}


def _build(n_cores=N_CORES, loop_n=1):
    nc = bacc.Bacc("TRN2", debug=False, num_devices=n_cores)

    xsT_d = nc.dram_tensor("xsT", (D, L), BF16, kind="ExternalInput")
    wqk_d = nc.dram_tensor("wqk_t", (128, 16, 8, 128), BF16,
                           kind="ExternalInput")
    wv_d = nc.dram_tensor("wv_t", (128, 2, 8, 512), BF16,
                          kind="ExternalInput")
    wo_d = nc.dram_tensor("wo_t", (128, 8, D), BF16, kind="ExternalInput")
    bqk_d = nc.dram_tensor("bqk_t", (128, 16), F32, kind="ExternalInput")
    bv_d = nc.dram_tensor("bv", (D,), F32, kind="ExternalInput")
    bo_d = nc.dram_tensor("bo", (D,), F32, kind="ExternalInput")
    out_d = nc.dram_tensor("out", (L, D), BF16, kind="ExternalOutput")

    with tile.TileContext(nc) as tc:
        if loop_n > 1:
            with tc.For_i(0, loop_n, 1):
                _emit(tc, out_d.ap(), xsT_d.ap(), wqk_d.ap(), wv_d.ap(),
                      wo_d.ap(), bqk_d.ap(), bv_d.ap(), bo_d.ap())
        else:
            _emit(tc, out_d.ap(), xsT_d.ap(), wqk_d.ap(), wv_d.ap(),
                  wo_d.ap(), bqk_d.ap(), bv_d.ap(), bo_d.ap())
    nc.compile()
    return nc


@with_exitstack
def _emit(ctx: ExitStack, tc, out, xsT_in, wqk, wv_in, wo_in, bqk_in, bv, bo):
    nc = tc.nc

    const_p = ctx.enter_context(tc.tile_pool(name="const", bufs=1))
    ctxT_p = ctx.enter_context(tc.tile_pool(name="ctxT", bufs=8))
    # filler projection matmuls share the attention score PSUM pool
    # (set once the attention block opens); PSUM is fully budgeted:
    # s_ps 4 banks + c_ps 4 banks.
    proj_pool = {}
    o_sb = ctx.enter_context(tc.tile_pool(name="o_sb", bufs=4))

    bqk = const_p.tile([128, 16], F32)
    bv_bc = const_p.tile([128, D], F32)
    bo_bc = const_p.tile([128, D], F32)

    ctxT = [ctxT_p.tile([128, L], BF16, tag="ctxT", name=f"ctxT{i}")
            for i in range(8)]

    with tc.tile_pool(name="xsT", bufs=1) as xsT_p, \
         tc.tile_pool(name="vaug", bufs=8) as vaug_p, \
         tc.tile_pool(name="qkT", bufs=4) as qkT_p, \
         tc.tile_pool(name="wqk", bufs=1) as wqk_p, \
         tc.tile_pool(name="wv", bufs=1) as wv_p, \
         tc.tile_pool(name="wout", bufs=1) as wo_p:

        xsT = xsT_p.tile([128, 8, L], BF16, tag="xsT", name="xsT")
        wqk_all = wqk_p.tile([128, 16, 8, 128], BF16, tag="wqk", name="wqk")
        wv_all = wv_p.tile([128, 2, 8, 512], BF16, tag="wv", name="wv")
        wo_all = wo_p.tile([128, 8, D], BF16, tag="wo", name="wo")

        # ---- input DMAs: one queue (sync), few large transfers ---------
        nc.sync.dma_start(out=bqk[:], in_=bqk_in[:, :])   # tiny, needed early
        nc.sync.dma_start(out=wqk_all[:, 0, :, :], in_=wqk[:, 0, :, :])
        nc.sync.dma_start(out=wqk_all[:, 8, :, :], in_=wqk[:, 8, :, :])
        for r in range(8):
            nc.sync.dma_start(out=xsT[:, r, :],
                              in_=xsT_in[r * 128:(r + 1) * 128, :])
        nc.sync.dma_start(out=wv_all[:, 0, :, :], in_=wv_in[:, 0, :, :])
        nc.sync.dma_start(out=wv_all[:, 1, :, :], in_=wv_in[:, 1, :, :])
        nc.sync.dma_start(out=wqk_all[:, 1:8, :, :], in_=wqk[:, 1:8, :, :])
        nc.sync.dma_start(out=wqk_all[:, 9:16, :, :],
                          in_=wqk[:, 9:16, :, :])
        nc.gpsimd.dma_start(out=bv_bc[:], in_=bv.partition_broadcast(128))
        nc.gpsimd.dma_start(out=bo_bc[:], in_=bo.partition_broadcast(128))

        # v_aug: per head a [128, 128] block: cols 0:64 = v dims, col 64 =
        # ones (softmax denominator), cols 65:128 dead (initialized but
        # never read back) — the full-128-col stationary enables FWL fast
        # weight loads on the ctx matmuls.  The memsets are emitted after
        # the chase evictions (below) so they don't delay them in the DVE
        # queue.
        vaug = [vaug_p.tile([128, H * 128], BF16, tag="vaug",
                            name=f"vaug{l}") for l in range(8)]

        # ---------- emission helpers --------------------------------------
        def emit_qk_tile(m, dest):
            """qkT row-tile m (dims m*128..) -> dest tile [128, L]."""
            units = []
            for half in range(2):
                def unit(half=half):
                    ps = proj_pool["p"].tile([128, 512], F32, tag="proj",
                                             name="ps")
                    for r in range(8):
                        nc.tensor.matmul(
                            ps[:], wqk_all[:, m, r, :],
                            xsT[:, r, half * 512:(half + 1) * 512],
                            start=(r == 0), stop=(r == 7),
                        )
                    if m >= 8:
                        # k: biasless ACT eviction (see chase note)
                        nc.scalar.activation(
                            out=dest[:, half * 512:(half + 1) * 512],
                            in_=ps[:], func=AF.Copy)
                    else:
                        nc.vector.tensor_scalar_add(
                            out=dest[:, half * 512:(half + 1) * 512],
                            in0=ps[:], scalar1=bqk[:, m:m + 1])
                units.append(unit)
            return units

        def emit_v_half(q):
            """v half q (heads 8q..8q+7) into vaug tiles; one unit per l."""
            units = []
            for l in range(8):
                def unit(l=l):
                    ps = proj_pool["p"].tile([128, 512], F32, tag="proj",
                                             name="vps")
                    for r in range(8):
                        nc.tensor.matmul(
                            ps[:], xsT[:, r, l * 128:(l + 1) * 128],
                            wv_all[:, q, r, :],
                            start=(r == 0), stop=(r == 7),
                        )
                    dst = vaug[l][:].rearrange("p (h e) -> p h e", e=128)
                    nc.vector.tensor_tensor(
                        out=dst[:, q * 8:(q + 1) * 8, HD:128],
                        in0=ps[:].rearrange("p (h e) -> p h e", e=HD),
                        in1=bv_bc[:].rearrange("p (h e) -> p h e", e=HD)[
                            :, q * 8:(q + 1) * 8, :],
                        op=mybir.AluOpType.add,
                    )
                units.append(unit)
            return units

        # ---- phase 0: pair-0 q/k chase over arriving xsT chunks ----------
        qk_tiles = {}
        qk_tiles[0] = (qkT_p.tile([128, L], BF16, tag="qkT", name="qt0"),
                       qkT_p.tile([128, L], BF16, tag="qkT", name="kt0"))
        with tc.tile_pool(name="chase_ps", bufs=4, space="PSUM") as ch_ps:
            chps = [ch_ps.tile([128, 512], F32, tag="ch", name=f"ch{i}")
                    for i in range(4)]
            for r in range(8):
                for half in range(2):
                    nc.tensor.matmul(
                        chps[half], wqk_all[:, 0, r, :],
                        xsT[:, r, half * 512:(half + 1) * 512],
                        start=(r == 0), stop=(r == 7))
                    nc.tensor.matmul(
                        chps[2 + half], wqk_all[:, 8, r, :],
                        xsT[:, r, half * 512:(half + 1) * 512],
                        start=(r == 0), stop=(r == 7))
            for half in range(2):
                nc.vector.tensor_scalar_add(
                    out=qk_tiles[0][0][:, half * 512:(half + 1) * 512],
                    in0=chps[half][:], scalar1=bqk[:, 0:1])
                # k tiles skip their bias: softmax is shift-invariant in the
                # query direction, so (q+bq)·k reproduces the reference
                # exactly while k·bk/bq·bk cancel in the softmax.
                nc.scalar.activation(
                    out=qk_tiles[0][1][:, half * 512:(half + 1) * 512],
                    in_=chps[2 + half][:], func=AF.Copy)

        for l in range(8):
            dst = vaug[l][:].rearrange("p (h e) -> p h e", e=128)
            nc.vector.memset(dst[:, :, 0:HD], 1.0)

        # ---- filler schedule ---------------------------------------------
        v0_units = emit_v_half(0)
        v1_units = []                  # created lazily at pair 1

        def build_pair_fillers(p):
            """Called at the first slot of pair p: append upcoming work."""
            units = []
            if p == 0:
                units += v0_units[2:]          # units 0/1 ran pre-loop
            if p <= 6:
                nxt = (qkT_p.tile([128, L], BF16, tag="qkT", name=f"qt{p+1}"),
                       qkT_p.tile([128, L], BF16, tag="qkT", name=f"kt{p+1}"))
                qk_tiles[p + 1] = nxt
                units += emit_qk_tile(p + 1, nxt[0])
                units += emit_qk_tile(9 + p, nxt[1])
            if p == 1:
                v1_units.extend(emit_v_half(1))
                units += v1_units[0:3]
            elif p == 2:
                units += v1_units[3:6]
            elif p == 3:
                units += v1_units[6:8]
            return units

        # per-slot filler counts: front-load v0 in pair 0 (vaug[c] must be
        # ready before ctx(0, 0, c) at slot c+LAG), then spread the rest.
        # slots are (pair, half, c): pair p covers slots 16p .. 16p+15.
        SLOTS = [(p, half, c) for p in range(H // 2) for half in range(2)
                 for c in range(8)]
        plan = [0] * len(SLOTS)
        for s in range(6):
            plan[s] = 1
        for s in (8, 10, 12, 14):
            plan[s] = 1
        for p in (1, 2, 3):
            base = 16 * p
            cnt = 7 if p < 3 else 6
            for i in range(cnt):
                plan[base + (i * 16) // cnt] = 1
        for p in (4, 5, 6):
            base = 16 * p
            for i in range(4):
                plan[base + i * 4] = 1

        # ---- attention: flat slot pipeline (wide eviction) ---------------
        # HW-measured mechanics this design is built around:
        #   * PSUM reads are the scarce resource (~550-614 ns per bank from
        #     ACT or DVE, and reads from different engines serialize on a
        #     shared path).  So each slot's score pair lands in ONE 2-bank
        #     [128,1024] PSUM tile and leaves through ONE wide ACT exp
        #     (~1.0 us) - the only PSUM read in the steady state.
        #   * A blocked semaphore wait costs ~300 ns to resume, so the PE
        #     (the busiest engine at ~1.5 us/slot incl. fillers) must never
        #     block: sps rotates 2 slots deep, et2 8 deep, ctx lags 5.
        with tc.tile_pool(name="et2", bufs=10) as et2_p, \
             tc.tile_pool(name="craw", bufs=6) as craw_p, \
             tc.tile_pool(name="rbc", bufs=4) as rbc_p, \
             tc.tile_pool(name="ps2", bufs=2, space="PSUM") as ps2_p, \
             tc.tile_pool(name="c_ps", bufs=2, space="PSUM") as c_ps, \
             tc.tile_pool(name="proj_ps", bufs=2, space="PSUM") as proj_ps:

            proj_pool["p"] = proj_ps
            # run two v units up front to cover the chase-eviction latency
            v0_units[0]()
            v0_units[1]()

            et_map = {}
            cps_map = {}
            fillers = []
            fidx = [0]

            def emit_sc(p, half, c):
                qt, kt = qk_tiles[p]
                hs = slice(half * 512, (half + 1) * 512)
                cb = slice(c * 128, (c + 1) * 128)
                sps = ps2_p.tile([128, 1024], F32, tag="sps2", name="sps")
                # both heads' scores concurrently in PE row groups 0-1 / 2-3
                nc.tensor.matmul(sps[:, 0:512], kt[0:HD, cb], qt[0:HD, hs],
                                 start=True, stop=True, tile_position=(0, 0))
                nc.tensor.matmul(sps[:, 512:1024], kt[HD:128, cb],
                                 qt[HD:128, hs], start=True, stop=True,
                                 tile_position=(HD, 0))
                et2 = et2_p.tile([128, 1024], BF16, tag="et2", name="et2")
                nc.scalar.activation(out=et2[:], in_=sps[:], func=AF.Exp,
                                     scale=SCALE)
                et_map[(p, half, c)] = et2

            norm_q = []

            def norm_stage_a(h, cps, hs, s):
                # vaug cols 0:64 are all-ones, so the ctx matmul leaves the
                # softmax denominator REPLICATED in cps partitions 0:64 (a
                # free PE partition-broadcast) and the values in 64:128.
                # The reciprocal reads the denominators straight from PSUM
                # (partition-aligned), so only the values need an ACT
                # eviction.  No gpsimd (HW: ~6 us/op), no single-partition
                # ops (HW: [1,512] reciprocal is ~3 us).
                craw = craw_p.tile([HD, 512], BF16, tag="craw", name="craw")
                nc.scalar.activation(out=craw[:], in_=cps[HD:128, :],
                                     func=AF.Copy)
                norm_q.append({"h": h, "hs": hs, "craw": craw, "cps": cps,
                               "stage": 1, "t": s})

            def norm_pump(s, force=False):
                for job in norm_q:
                    if job["stage"] == 1 and (force or s - job["t"] >= 2):
                        rec = rbc_p.tile([HD, 512], F32, tag="rec",
                                         name="rec")
                        nc.vector.reciprocal_approx_fast(
                            out=rec[:], in_=job["cps"][0:HD, :])
                        job["rec"] = rec
                        job["stage"] = 2
                        job["t"] = s
                    elif job["stage"] == 2 and (force or s - job["t"] >= 2):
                        h, po = job["h"], (job["h"] % 2) * HD
                        nc.vector.tensor_tensor(
                            out=ctxT[h // 2][po:po + HD, job["hs"]],
                            in0=job["craw"][:],
                            in1=job["rec"][:],
                            op=mybir.AluOpType.mult)
                        job["stage"] = 3
                norm_q[:] = [j for j in norm_q if j["stage"] < 3]

            def emit_ctx(p, half, c):
                if c == 0:
                    cps_map[(p, half)] = (
                        c_ps.tile([128, 512], F32, tag="cps", name="cpsA"),
                        c_ps.tile([128, 512], F32, tag="cps", name="cpsB"))
                cpsA, cpsB = cps_map[(p, half)]
                et2 = et_map.pop((p, half, c))
                hs = slice(half * 512, (half + 1) * 512)
                hA, hB = 2 * p, 2 * p + 1
                nc.tensor.matmul(
                    cpsA[:], vaug[c][:, hA * 128:(hA + 1) * 128],
                    et2[:, 0:512], start=(c == 0), stop=(c == 7))
                nc.tensor.matmul(
                    cpsB[:], vaug[c][:, hB * 128:(hB + 1) * 128],
                    et2[:, 512:1024], start=(c == 0), stop=(c == 7))
                if c == 7:
                    s = 16 * p + 8 * half + 7 + LAG
                    norm_stage_a(hA, cpsA, hs, s)
                    norm_stage_a(hB, cpsB, hs, s)
                    cps_map.pop((p, half))

            for s, (p, half, c) in enumerate(SLOTS):
                if half == 0 and c == 0:
                    fillers += build_pair_fillers(p)
                if s == 40:
                    # wout load: single 2MB DMA, well before the out phase
                    nc.sync.dma_start(out=wo_all[:], in_=wo_in[:, :, :])
                emit_sc(p, half, c)
                for _ in range(plan[s]):
                    if fidx[0] < len(fillers):
                        fillers[fidx[0]]()
                        fidx[0] += 1
                if s >= LAG:
                    emit_ctx(*SLOTS[s - LAG])
                norm_pump(s)
            while fidx[0] < len(fillers):   # safety drain (should be empty)
                fillers[fidx[0]]()
                fidx[0] += 1

            # ---- drain + out-proj lead-in --------------------------------
            # The final LAG ctx emissions are interleaved with the first
            # two out units' r=0..6 matmuls (emitted BEFORE the pair-7
            # normalize so their semaphore waits exclude it); only r=7
            # contracts ctxT[7], so the PE keeps working while the pair-7
            # normalize chain finishes.
            def ounit_mm(ps, l, half, rs):
                for r in rs:
                    nc.tensor.matmul(
                        ps[:], ctxT[r][:, l * 128:(l + 1) * 128],
                        wo_all[:, r, half * 512:(half + 1) * 512],
                        start=(r == 0), stop=(r == 7),
                    )

            def ounit_fin(ps, l, half):
                osb = o_sb.tile([128, 512], BF16, tag="osb", name="osb")
                nc.vector.tensor_tensor(
                    out=osb[:], in0=ps[:],
                    in1=bo_bc[:, half * 512:(half + 1) * 512],
                    op=mybir.AluOpType.add)
                nc.sync.dma_start(
                    out=out[l * 128:(l + 1) * 128,
                            half * 512:(half + 1) * 512],
                    in_=osb[:],
                )

            drain = SLOTS[len(SLOTS) - LAG:]
            ps_a = proj_ps.tile([128, 512], F32, tag="proj", name="opsa")
            sps_b = ps2_p.tile([128, 1024], F32, tag="sps2", name="opsb")
            ps_b = sps_b[:, 0:512]
            emit_ctx(*drain[0])
            emit_ctx(*drain[1])
            norm_pump(128)
            ounit_mm(ps_a, 0, 0, range(0, 4))
            emit_ctx(*drain[2])
            norm_pump(130)
            ounit_mm(ps_a, 0, 0, range(4, 7))
            ounit_mm(ps_b, 0, 1, range(0, 3))
            emit_ctx(*drain[3])
            emit_ctx(*drain[4])        # creates the pair-7 half-1 chains
            norm_pump(132)
            ounit_mm(ps_b, 0, 1, range(3, 7))
            norm_pump(134, force=True)
            ounit_mm(ps_a, 0, 0, [7])
            norm_pump(136, force=True)
            ounit_fin(ps_a, 0, 0)
            norm_pump(138, force=True)
            assert not norm_q
            ounit_mm(ps_b, 0, 1, [7])
            ounit_fin(ps_b, 0, 1)

        # ---- phase 3: remaining out units (attention PSUM now free) ------
        with tc.tile_pool(name="o_ps", bufs=3, space="PSUM") as o_ps:
            for l in range(1, 8):
                for half in range(2):
                    ps = o_ps.tile([128, 512], F32, tag="ops", name="ops")
                    ounit_mm(ps, l, half, range(8))
                    ounit_fin(ps, l, half)


def get_nc():
    if "nc" not in _CACHE:
        _CACHE["nc"] = _build()
    return _CACHE["nc"]


def _prep_weights(Wqkv, bqkv, Wout, bout):
    if "w" not in _CACHE:
        Wqkv = np.asarray(Wqkv, dtype=np.float32)
        wqk = Wqkv[:, :NQK]                      # [1024, 2048]
        # wqk_t[p, m, r, c] = Wqkv[r*128+p, m*128+c]
        wqk_t = np.ascontiguousarray(
            wqk.reshape(8, 128, 16, 128).transpose(1, 2, 0, 3)
        ).astype(ml_dtypes.bfloat16)
        wvn = Wqkv[:, NQK:]                      # [1024, 1024]
        # wv_t[p, q, r, n] = Wqkv[r*128+p, 2048 + q*512 + n]
        wv_t = np.ascontiguousarray(
            wvn.reshape(8, 128, 2, 512).transpose(1, 2, 0, 3)
        ).astype(ml_dtypes.bfloat16)
        # wo_t[p, r, n] = Wout[r*128+p, n]
        wo_t = np.ascontiguousarray(
            np.asarray(Wout, dtype=np.float32).reshape(8, 128, D)
            .transpose(1, 0, 2)).astype(ml_dtypes.bfloat16)
        bqk_t = np.ascontiguousarray(
            np.asarray(bqkv[:NQK], dtype=np.float32).reshape(16, 128).T)
        bv = np.ascontiguousarray(np.asarray(bqkv[NQK:], dtype=np.float32))
        bo = np.ascontiguousarray(np.asarray(bout, dtype=np.float32))
        _CACHE["w"] = dict(wqk_t=wqk_t, wv_t=wv_t, wo_t=wo_t, bqk_t=bqk_t,
                           bv=bv, bo=bo)
    return _CACHE["w"]


def make_in_maps(x, Wqkv, bqkv, Wout, bout):
    """Shard: core i -> (batch i//N_SEG, segment i%N_SEG), dilated tokens.

    All layout/dtype prep happens host-side: xs is transposed to [D, L]
    and cast to bf16; weights are tiled so each DMA reads contiguous
    multi-KB per-partition lines.
    """
    w = _prep_weights(Wqkv, bqkv, Wout, bout)
    x = np.asarray(x, dtype=np.float32)
    in_maps = []
    for i in range(N_CORES):
        b, seg = divmod(i, N_SEG)
        xs = x[b, seg * SEGMENT:(seg + 1) * SEGMENT:DILATION, :]
        xsT = np.ascontiguousarray(xs.T).astype(ml_dtypes.bfloat16)
        in_maps.append({"xsT": xsT, **w})
    return in_maps


def unshard(results):
    out = np.empty((B, N_SEG * L, D), dtype=np.float32)
    for i in range(N_CORES):
        b, seg = divmod(i, N_SEG)
        out[b, seg * L:(seg + 1) * L, :] = np.asarray(
            results[i]["out"], dtype=np.float32)
    return out


def kernel(x, Wqkv, bqkv, Wout, bout):
    nc = get_nc()
    in_maps = make_in_maps(x, Wqkv, bqkv, Wout, bout)
    res = bass_utils.run_bass_kernel_spmd(nc, in_maps,
                                          core_ids=list(range(N_CORES)))
    return unshard(res.results)



# revision 29
# speedup vs baseline: 4.2050x; 4.2050x over previous
"""Dilated-attention (segmented FlashMHA) for Trainium2, 8-core data parallel.

Problem (hardcoded): x [2, 8192, 1024], SEGMENT=2048, DILATION=2, 16 heads.
Each (batch, segment) pair is an independent attention problem over the
L = 1024 dilated tokens; there are exactly B * n_seg = 2 * 4 = 8 of them,
one per NeuronCore.  Weights are replicated.

v7: the attention pipeline is rebuilt around three
HW-measured facts that the cost model misses:
  1. PSUM reads are the scarce resource: ~550-614 ns per [128,512] f32
     bank from ACT or DVE, and reads from *different* engines serialize
     on a shared path (splitting evictions across engines gains nothing).
     So each slot's score pair lands in ONE 2-bank [128,1024] PSUM tile
     and leaves through ONE wide ACT exp (~1.0 us) - ACT is the only
     PSUM reader in the attention steady state.
  2. A blocked semaphore wait costs ~300 ns to resume, so the busiest
     engine must never block.  The PE (~1.5 us/slot incl. fillers) runs
     against a 2-slot sps window (ps2 bufs=2), 5-slot ctx lag, 10-deep
     et2 pool; ACT and DVE absorb the resume penalties instead.
  3. gpsimd (Pool) ops cost ~6 us each and a [1,512] single-partition
     reciprocal ~3 us - both are banished from the softmax normalize.
     vaug columns 64:128 are all-ones, so the ctx matmul replicates the
     softmax denominator across cps partitions 64:128 for free (a PE
     partition-broadcast); normalize is then: one bf16 ACT eviction of
     the values + one f32 ACT eviction of the denominators + DVE
     reciprocal_approx_fast + one DVE mult, pumped >=2 slots apart so
     nothing ever waits.
  k-tiles skip their bias exactly (softmax is shift-invariant along the
  query axis: only (q+bq)-k survives; k-bk and bq-bk cancel), which
  turns their eviction into a biasless ACT copy.

Per-core kernel phases:
  chase   pair-0 q/k tiles accumulate as xsT DMA chunks land
  attn    128 half-slots (pair, half, c): score pair (row-group paired
          K=64 matmuls, ~161 ns) -> wide exp -> 2 ctx matmuls, with the
          qk/v projection units interleaved as PE fillers per `plan`
  out     ctxT.T-contract @ Wout + bout, lead-in interleaved with the
          drain so the PE never idles on the last normalize
"""

from contextlib import ExitStack

import numpy as np
import ml_dtypes

from concourse import bacc, bass_utils, mybir, tile
from concourse._compat import with_exitstack

F32 = mybir.dt.float32
BF16 = mybir.dt.bfloat16
AF = mybir.ActivationFunctionType

B = 2
S = 8192
D = 1024
SEGMENT = 2048
DILATION = 2
N_SEG = S // SEGMENT          # 4
L = SEGMENT // DILATION       # 1024 tokens per (b, seg)
H = 16
HD = 64
NQK = 2048
SCALE = 0.125                 # 1 / sqrt(HD)
N_CORES = 8
LAG = 5                       # ctx trails scores by LAG half-slots
L1 = 2                        # head-B exp (from SBUF) trails scores by L1

_CACHE = {}


def _build(n_cores=N_CORES, loop_n=1):
    nc = bacc.Bacc("TRN2", debug=False, num_devices=n_cores)

    xsT_d = nc.dram_tensor("xsT", (D, L), BF16, kind="ExternalInput")
    wqk_d = nc.dram_tensor("wqk_t", (128, 16, 8, 128), BF16,
                           kind="ExternalInput")
    wv_d = nc.dram_tensor("wv_t", (128, 2, 8, 512), BF16,
                          kind="ExternalInput")
    wo_d = nc.dram_tensor("wo_t", (128, 8, D), BF16, kind="ExternalInput")
    bqk_d = nc.dram_tensor("bqk_t", (128, 16), F32, kind="ExternalInput")
    bv_d = nc.dram_tensor("bv", (D,), F32, kind="ExternalInput")
    bo_d = nc.dram_tensor("bo", (D,), F32, kind="ExternalInput")
    out_d = nc.dram_tensor("out", (L, D), BF16, kind="ExternalOutput")

    with tile.TileContext(nc) as tc:
        if loop_n > 1:
            with tc.For_i(0, loop_n, 1):
                _emit(tc, out_d.ap(), xsT_d.ap(), wqk_d.ap(), wv_d.ap(),
                      wo_d.ap(), bqk_d.ap(), bv_d.ap(), bo_d.ap())
        else:
            _emit(tc, out_d.ap(), xsT_d.ap(), wqk_d.ap(), wv_d.ap(),
                  wo_d.ap(), bqk_d.ap(), bv_d.ap(), bo_d.ap())
    nc.compile()
    return nc


@with_exitstack
def _emit(ctx: ExitStack, tc, out, xsT_in, wqk, wv_in, wo_in, bqk_in, bv, bo):
    nc = tc.nc

    const_p = ctx.enter_context(tc.tile_pool(name="const", bufs=1))
    ctxT_p = ctx.enter_context(tc.tile_pool(name="ctxT", bufs=8))
    # filler projection matmuls share the attention score PSUM pool
    # (set once the attention block opens); PSUM is fully budgeted:
    # s_ps 4 banks + c_ps 4 banks.
    proj_pool = {}
    o_sb = ctx.enter_context(tc.tile_pool(name="o_sb", bufs=4))

    bqk = const_p.tile([128, 16], F32)
    bv_bc = const_p.tile([128, D], F32)
    bo_bc = const_p.tile([128, D], F32)

    ctxT = [ctxT_p.tile([128, L], BF16, tag="ctxT", name=f"ctxT{i}")
            for i in range(8)]

    with tc.tile_pool(name="xsT", bufs=1) as xsT_p, \
         tc.tile_pool(name="vaug", bufs=8) as vaug_p, \
         tc.tile_pool(name="qkT", bufs=4) as qkT_p, \
         tc.tile_pool(name="wqk", bufs=1) as wqk_p, \
         tc.tile_pool(name="wv", bufs=1) as wv_p, \
         tc.tile_pool(name="wout", bufs=1) as wo_p:

        xsT = xsT_p.tile([128, 8, L], BF16, tag="xsT", name="xsT")
        wqk_all = wqk_p.tile([128, 16, 8, 128], BF16, tag="wqk", name="wqk")
        wv_all = wv_p.tile([128, 2, 8, 512], BF16, tag="wv", name="wv")
        wo_all = wo_p.tile([128, 8, D], BF16, tag="wo", name="wo")

        # ---- input DMAs: one queue (sync), few large transfers ---------
        nc.sync.dma_start(out=bqk[:], in_=bqk_in[:, :])   # tiny, needed early
        nc.sync.dma_start(out=wqk_all[:, 0, :, :], in_=wqk[:, 0, :, :])
        nc.sync.dma_start(out=wqk_all[:, 8, :, :], in_=wqk[:, 8, :, :])
        for r in range(8):
            nc.sync.dma_start(out=xsT[:, r, :],
                              in_=xsT_in[r * 128:(r + 1) * 128, :])
        nc.sync.dma_start(out=wv_all[:, 0, :, :], in_=wv_in[:, 0, :, :])
        nc.sync.dma_start(out=wv_all[:, 1, :, :], in_=wv_in[:, 1, :, :])
        nc.sync.dma_start(out=wqk_all[:, 1:8, :, :], in_=wqk[:, 1:8, :, :])
        nc.sync.dma_start(out=wqk_all[:, 9:16, :, :],
                          in_=wqk[:, 9:16, :, :])
        nc.gpsimd.dma_start(out=bv_bc[:], in_=bv.partition_broadcast(128))
        nc.gpsimd.dma_start(out=bo_bc[:], in_=bo.partition_broadcast(128))

        # v_aug: per head a [128, 128] block: cols 0:64 = v dims, col 64 =
        # ones (softmax denominator), cols 65:128 dead (initialized but
        # never read back) — the full-128-col stationary enables FWL fast
        # weight loads on the ctx matmuls.  The memsets are emitted after
        # the chase evictions (below) so they don't delay them in the DVE
        # queue.
        vaug = [vaug_p.tile([128, H * 128], BF16, tag="vaug",
                            name=f"vaug{l}") for l in range(8)]

        # ---------- emission helpers --------------------------------------
        def emit_qk_tile(m, dest):
            """qkT row-tile m (dims m*128..) -> dest tile [128, L]."""
            units = []
            for half in range(2):
                def unit(half=half):
                    ps = proj_pool["p"].tile([128, 512], F32, tag="proj",
                                             name="ps")
                    for r in range(8):
                        nc.tensor.matmul(
                            ps[:], wqk_all[:, m, r, :],
                            xsT[:, r, half * 512:(half + 1) * 512],
                            start=(r == 0), stop=(r == 7),
                        )
                    if m >= 8:
                        # k: biasless ACT eviction (see chase note)
                        nc.scalar.activation(
                            out=dest[:, half * 512:(half + 1) * 512],
                            in_=ps[:], func=AF.Copy)
                    else:
                        nc.vector.tensor_scalar_add(
                            out=dest[:, half * 512:(half + 1) * 512],
                            in0=ps[:], scalar1=bqk[:, m:m + 1])
                units.append(unit)
            return units

        def emit_v_half(q):
            """v half q (heads 8q..8q+7) into vaug tiles; one unit per l."""
            units = []
            for l in range(8):
                def unit(l=l):
                    ps = proj_pool["p"].tile([128, 512], F32, tag="proj",
                                             name="vps")
                    for r in range(8):
                        nc.tensor.matmul(
                            ps[:], xsT[:, r, l * 128:(l + 1) * 128],
                            wv_all[:, q, r, :],
                            start=(r == 0), stop=(r == 7),
                        )
                    dst = vaug[l][:].rearrange("p (h e) -> p h e", e=128)
                    nc.vector.tensor_tensor(
                        out=dst[:, q * 8:(q + 1) * 8, 0:HD],
                        in0=ps[:].rearrange("p (h e) -> p h e", e=HD),
                        in1=bv_bc[:].rearrange("p (h e) -> p h e", e=HD)[
                            :, q * 8:(q + 1) * 8, :],
                        op=mybir.AluOpType.add,
                    )
                units.append(unit)
            return units

        # ---- phase 0: pair-0 q/k chase over arriving xsT chunks ----------
        qk_tiles = {}
        qk_tiles[0] = (qkT_p.tile([128, L], BF16, tag="qkT", name="qt0"),
                       qkT_p.tile([128, L], BF16, tag="qkT", name="kt0"))
        with tc.tile_pool(name="chase_ps", bufs=4, space="PSUM") as ch_ps:
            chps = [ch_ps.tile([128, 512], F32, tag="ch", name=f"ch{i}")
                    for i in range(4)]
            for r in range(8):
                for half in range(2):
                    nc.tensor.matmul(
                        chps[half], wqk_all[:, 0, r, :],
                        xsT[:, r, half * 512:(half + 1) * 512],
                        start=(r == 0), stop=(r == 7))
                    nc.tensor.matmul(
                        chps[2 + half], wqk_all[:, 8, r, :],
                        xsT[:, r, half * 512:(half + 1) * 512],
                        start=(r == 0), stop=(r == 7))
            for half in range(2):
                nc.vector.tensor_scalar_add(
                    out=qk_tiles[0][0][:, half * 512:(half + 1) * 512],
                    in0=chps[half][:], scalar1=bqk[:, 0:1])
                # k tiles skip their bias: softmax is shift-invariant in the
                # query direction, so (q+bq)·k reproduces the reference
                # exactly while k·bk/bq·bk cancel in the softmax.
                nc.scalar.activation(
                    out=qk_tiles[0][1][:, half * 512:(half + 1) * 512],
                    in_=chps[2 + half][:], func=AF.Copy)

        for l in range(8):
            dst = vaug[l][:].rearrange("p (h e) -> p h e", e=128)
            nc.vector.memset(dst[:, :, HD:128], 1.0)

        # ---- filler schedule ---------------------------------------------
        v0_units = emit_v_half(0)
        v1_units = []                  # created lazily at pair 1

        def build_pair_fillers(p):
            """Called at the first slot of pair p: append upcoming work."""
            units = []
            if p == 0:
                units += v0_units[2:]          # units 0/1 ran pre-loop
            if p <= 6:
                nxt = (qkT_p.tile([128, L], BF16, tag="qkT", name=f"qt{p+1}"),
                       qkT_p.tile([128, L], BF16, tag="qkT", name=f"kt{p+1}"))
                qk_tiles[p + 1] = nxt
                units += emit_qk_tile(p + 1, nxt[0])
                units += emit_qk_tile(9 + p, nxt[1])
            if p == 1:
                v1_units.extend(emit_v_half(1))
                units += v1_units[0:3]
            elif p == 2:
                units += v1_units[3:6]
            elif p == 3:
                units += v1_units[6:8]
            return units

        # per-slot filler counts: front-load v0 in pair 0 (vaug[c] must be
        # ready before ctx(0, 0, c) at slot c+LAG), then spread the rest.
        # slots are (pair, half, c): pair p covers slots 16p .. 16p+15.
        SLOTS = [(p, half, c) for p in range(H // 2) for half in range(2)
                 for c in range(8)]
        plan = [0] * len(SLOTS)
        for s in range(6):
            plan[s] = 1
        for s in (8, 10, 12, 14):
            plan[s] = 1
        for p in (1, 2, 3):
            base = 16 * p
            cnt = 7 if p < 3 else 6
            for i in range(cnt):
                plan[base + (i * 16) // cnt] = 1
        for p in (4, 5, 6):
            base = 16 * p
            for i in range(4):
                plan[base + i * 4] = 1

        # ---- attention: flat slot pipeline (wide eviction) ---------------
        # HW-measured mechanics this design is built around:
        #   * PSUM reads are the scarce resource (~550-614 ns per bank from
        #     ACT or DVE, and reads from different engines serialize on a
        #     shared path).  So each slot's score pair lands in ONE 2-bank
        #     [128,1024] PSUM tile and leaves through ONE wide ACT exp
        #     (~1.0 us) - the only PSUM read in the steady state.
        #   * A blocked semaphore wait costs ~300 ns to resume, so the PE
        #     (the busiest engine at ~1.5 us/slot incl. fillers) must never
        #     block: sps rotates 2 slots deep, et2 8 deep, ctx lags 5.
        with tc.tile_pool(name="et2", bufs=10) as et2_p, \
             tc.tile_pool(name="craw", bufs=6) as craw_p, \
             tc.tile_pool(name="rbc", bufs=4) as rbc_p, \
             tc.tile_pool(name="ps2", bufs=2, space="PSUM") as ps2_p, \
             tc.tile_pool(name="c_ps", bufs=2, space="PSUM") as c_ps, \
             tc.tile_pool(name="proj_ps", bufs=2, space="PSUM") as proj_ps:

            proj_pool["p"] = proj_ps
            # run two v units up front to cover the chase-eviction latency
            v0_units[0]()
            v0_units[1]()

            et_map = {}
            cps_map = {}
            fillers = []
            fidx = [0]

            def emit_sc(p, half, c):
                qt, kt = qk_tiles[p]
                hs = slice(half * 512, (half + 1) * 512)
                cb = slice(c * 128, (c + 1) * 128)
                sps = ps2_p.tile([128, 1024], F32, tag="sps2", name="sps")
                # both heads' scores concurrently in PE row groups 0-1 / 2-3
                nc.tensor.matmul(sps[:, 0:512], kt[0:HD, cb], qt[0:HD, hs],
                                 start=True, stop=True, tile_position=(0, 0))
                nc.tensor.matmul(sps[:, 512:1024], kt[HD:128, cb],
                                 qt[HD:128, hs], start=True, stop=True,
                                 tile_position=(HD, 0))
                et2 = et2_p.tile([128, 1024], BF16, tag="et2", name="et2")
                nc.scalar.activation(out=et2[:], in_=sps[:], func=AF.Exp,
                                     scale=SCALE)
                et_map[(p, half, c)] = et2

            norm_q = []

            def norm_stage_a(h, cps, hs, s):
                # vaug cols 64:128 are all-ones, so the ctx matmul already
                # left the softmax denominator REPLICATED in cps partitions
                # 64:128 -- a free partition-broadcast done by the PE.  One
                # f32 eviction frees the bank; reciprocal_approx_fast (~5x
                # faster than InstReciprocal, 18-bit) and one mult finish
                # the job.  No gpsimd (HW: each Pool op costs ~6 us) and no
                # single-partition ops (HW: [1,512] reciprocal is ~3 us).
                craw = craw_p.tile([HD, 512], BF16, tag="craw", name="craw")
                nc.scalar.activation(out=craw[:], in_=cps[0:HD, :],
                                     func=AF.Copy)
                den = craw_p.tile([HD, 512], F32, tag="den", name="den")
                nc.scalar.activation(out=den[:], in_=cps[HD:128, :],
                                     func=AF.Copy)
                norm_q.append({"h": h, "hs": hs, "craw": craw, "den": den,
                               "stage": 1, "t": s})

            def norm_pump(s, force=False):
                for job in norm_q:
                    if job["stage"] == 1 and (force or s - job["t"] >= 2):
                        rec = rbc_p.tile([HD, 512], F32, tag="rec",
                                         name="rec")
                        nc.vector.reciprocal_approx_fast(
                            out=rec[:], in_=job["den"][:])
                        job["rec"] = rec
                        job["stage"] = 2
                        job["t"] = s
                    elif job["stage"] == 2 and (force or s - job["t"] >= 2):
                        h, po = job["h"], (job["h"] % 2) * HD
                        nc.vector.tensor_tensor(
                            out=ctxT[h // 2][po:po + HD, job["hs"]],
                            in0=job["craw"][:],
                            in1=job["rec"][:],
                            op=mybir.AluOpType.mult)
                        job["stage"] = 3
                norm_q[:] = [j for j in norm_q if j["stage"] < 3]

            def emit_ctx(p, half, c):
                if c == 0:
                    cps_map[(p, half)] = (
                        c_ps.tile([128, 512], F32, tag="cps", name="cpsA"),
                        c_ps.tile([128, 512], F32, tag="cps", name="cpsB"))
                cpsA, cpsB = cps_map[(p, half)]
                et2 = et_map.pop((p, half, c))
                hs = slice(half * 512, (half + 1) * 512)
                hA, hB = 2 * p, 2 * p + 1
                nc.tensor.matmul(
                    cpsA[:], vaug[c][:, hA * 128:(hA + 1) * 128],
                    et2[:, 0:512], start=(c == 0), stop=(c == 7))
                nc.tensor.matmul(
                    cpsB[:], vaug[c][:, hB * 128:(hB + 1) * 128],
                    et2[:, 512:1024], start=(c == 0), stop=(c == 7))
                if c == 7:
                    s = 16 * p + 8 * half + 7 + LAG
                    norm_stage_a(hA, cpsA, hs, s)
                    norm_stage_a(hB, cpsB, hs, s)
                    cps_map.pop((p, half))

            for s, (p, half, c) in enumerate(SLOTS):
                if half == 0 and c == 0:
                    fillers += build_pair_fillers(p)
                if s == 40:
                    # wout load: single 2MB DMA, well before the out phase
                    nc.sync.dma_start(out=wo_all[:], in_=wo_in[:, :, :])
                emit_sc(p, half, c)
                for _ in range(plan[s]):
                    if fidx[0] < len(fillers):
                        fillers[fidx[0]]()
                        fidx[0] += 1
                if s >= LAG:
                    emit_ctx(*SLOTS[s - LAG])
                norm_pump(s)
            while fidx[0] < len(fillers):   # safety drain (should be empty)
                fillers[fidx[0]]()
                fidx[0] += 1

            # ---- drain + out-proj lead-in --------------------------------
            # The final LAG ctx emissions are interleaved with the first
            # two out units' r=0..6 matmuls (emitted BEFORE the pair-7
            # normalize so their semaphore waits exclude it); only r=7
            # contracts ctxT[7], so the PE keeps working while the pair-7
            # normalize chain finishes.
            def ounit_mm(ps, l, half, rs):
                for r in rs:
                    nc.tensor.matmul(
                        ps[:], ctxT[r][:, l * 128:(l + 1) * 128],
                        wo_all[:, r, half * 512:(half + 1) * 512],
                        start=(r == 0), stop=(r == 7),
                    )

            def ounit_fin(ps, l, half):
                osb = o_sb.tile([128, 512], BF16, tag="osb", name="osb")
                nc.vector.tensor_tensor(
                    out=osb[:], in0=ps[:],
                    in1=bo_bc[:, half * 512:(half + 1) * 512],
                    op=mybir.AluOpType.add)
                nc.sync.dma_start(
                    out=out[l * 128:(l + 1) * 128,
                            half * 512:(half + 1) * 512],
                    in_=osb[:],
                )

            drain = SLOTS[len(SLOTS) - LAG:]
            ps_a = proj_ps.tile([128, 512], F32, tag="proj", name="opsa")
            sps_b = ps2_p.tile([128, 1024], F32, tag="sps2", name="opsb")
            ps_b = sps_b[:, 0:512]
            emit_ctx(*drain[0])
            emit_ctx(*drain[1])
            norm_pump(128)
            ounit_mm(ps_a, 0, 0, range(0, 4))
            emit_ctx(*drain[2])
            norm_pump(130)
            ounit_mm(ps_a, 0, 0, range(4, 7))
            ounit_mm(ps_b, 0, 1, range(0, 3))
            emit_ctx(*drain[3])
            emit_ctx(*drain[4])        # creates the pair-7 half-1 chains
            norm_pump(132)
            ounit_mm(ps_b, 0, 1, range(3, 7))
            norm_pump(134, force=True)
            ounit_mm(ps_a, 0, 0, [7])
            norm_pump(136, force=True)
            ounit_fin(ps_a, 0, 0)
            norm_pump(138, force=True)
            assert not norm_q
            ounit_mm(ps_b, 0, 1, [7])
            ounit_fin(ps_b, 0, 1)

        # ---- phase 3: remaining out units (attention PSUM now free) ------
        with tc.tile_pool(name="o_ps", bufs=3, space="PSUM") as o_ps:
            for l in range(1, 8):
                for half in range(2):
                    ps = o_ps.tile([128, 512], F32, tag="ops", name="ops")
                    ounit_mm(ps, l, half, range(8))
                    ounit_fin(ps, l, half)


def get_nc():
    if "nc" not in _CACHE:
        _CACHE["nc"] = _build()
    return _CACHE["nc"]


def _prep_weights(Wqkv, bqkv, Wout, bout):
    if "w" not in _CACHE:
        Wqkv = np.asarray(Wqkv, dtype=np.float32)
        wqk = Wqkv[:, :NQK]                      # [1024, 2048]
        # wqk_t[p, m, r, c] = Wqkv[r*128+p, m*128+c]
        wqk_t = np.ascontiguousarray(
            wqk.reshape(8, 128, 16, 128).transpose(1, 2, 0, 3)
        ).astype(ml_dtypes.bfloat16)
        wvn = Wqkv[:, NQK:]                      # [1024, 1024]
        # wv_t[p, q, r, n] = Wqkv[r*128+p, 2048 + q*512 + n]
        wv_t = np.ascontiguousarray(
            wvn.reshape(8, 128, 2, 512).transpose(1, 2, 0, 3)
        ).astype(ml_dtypes.bfloat16)
        # wo_t[p, r, n] = Wout[r*128+p, n]
        wo_t = np.ascontiguousarray(
            np.asarray(Wout, dtype=np.float32).reshape(8, 128, D)
            .transpose(1, 0, 2)).astype(ml_dtypes.bfloat16)
        bqk_t = np.ascontiguousarray(
            np.asarray(bqkv[:NQK], dtype=np.float32).reshape(16, 128).T)
        bv = np.ascontiguousarray(np.asarray(bqkv[NQK:], dtype=np.float32))
        bo = np.ascontiguousarray(np.asarray(bout, dtype=np.float32))
        _CACHE["w"] = dict(wqk_t=wqk_t, wv_t=wv_t, wo_t=wo_t, bqk_t=bqk_t,
                           bv=bv, bo=bo)
    return _CACHE["w"]


def make_in_maps(x, Wqkv, bqkv, Wout, bout):
    """Shard: core i -> (batch i//N_SEG, segment i%N_SEG), dilated tokens.

    All layout/dtype prep happens host-side: xs is transposed to [D, L]
    and cast to bf16; weights are tiled so each DMA reads contiguous
    multi-KB per-partition lines.
    """
    w = _prep_weights(Wqkv, bqkv, Wout, bout)
    x = np.asarray(x, dtype=np.float32)
    in_maps = []
    for i in range(N_CORES):
        b, seg = divmod(i, N_SEG)
        xs = x[b, seg * SEGMENT:(seg + 1) * SEGMENT:DILATION, :]
        xsT = np.ascontiguousarray(xs.T).astype(ml_dtypes.bfloat16)
        in_maps.append({"xsT": xsT, **w})
    return in_maps


def unshard(results):
    out = np.empty((B, N_SEG * L, D), dtype=np.float32)
    for i in range(N_CORES):
        b, seg = divmod(i, N_SEG)
        out[b, seg * L:(seg + 1) * L, :] = np.asarray(
            results[i]["out"], dtype=np.float32)
    return out


def kernel(x, Wqkv, bqkv, Wout, bout):
    nc = get_nc()
    in_maps = make_in_maps(x, Wqkv, bqkv, Wout, bout)
    res = bass_utils.run_bass_kernel_spmd(nc, in_maps,
                                          core_ids=list(range(N_CORES)))
    return unshard(res.results)

